# revision 52
# baseline (speedup 1.0000x reference)
"""MoE block (AdaptFormer adapters, top-2 of 8 experts) on 8 TRN2 NeuronCores.

Data-parallel over the 8192 tokens (1024/core), router + expert adapter
weights replicated. Per core:
  - x ships as an exact bf16 hi/lo split (xh + xl == x to 2^-17),
    pre-transposed and chunk-pair-packed on the host so every big input
    lands in one 512KB DMA (DMA issue costs ~650ns/instruction on the
    sync ring regardless of size — fewer, fatter DMAs). Ring order:
    wd, xh, xl, wu.
  - The expert path uses xh only and runs entirely in bf16 (measured
    end-to-end rel L2 err ~4e-3 vs the 2e-2 gate); xl exists for the
    router only.
  - logits = x @ w_gate exactly enough (error ~3e-6, far below the
    3.6e-5 min top-2/3 logit gap) via three bf16 accumulation passes
    xh@wgh + xh@wgl + xl@wgh. Terms 1+2 and GEMM1's k=0 pass chase the
    xh stream chunk-by-chunk; term 3 chases xl.
  - PE warm-up + filler matmuls keep the HAM clock gate at K=8/8
    (2.4 GHz); an idle PE re-throttles to 4/8 (half rate).
  - gating: per 128-token tile, PE-transpose logits, top-2 softmax
    (x0.5) on DVE/ACT, cast bf16, PE-transpose back to g2T [8, tok].
    Remaining HT passes interleave so the PE never waits on a chain.
  - GEMM1: HT chunks [128-of-512, tok] = Wd_chunk^T @ xh in bf16.
    Gate row GB = Eblk_chunk^T @ g2T (matmul), then
    HG = max(HT, 0) * GB fused in one DVE scalar_tensor_tensor.
  - GEMM2: out tiles [128 tok, 1024] = HG-slices @ Wu accumulated over
    the expert axis; stored bf16 (host converts to f32).
All experts computed densely; sparse gates zero the non-top-2 terms.
"""
import numpy as np
import ml_dtypes
from contextlib import ExitStack

import concourse.bass as bass
import concourse.tile as tile
from concourse.tile import add_dep_helper
from concourse import bacc, mybir
from concourse.bass_utils import run_bass_kernel_spmd

N_CORES = 8
B_DIM, S_DIM, D = 2, 4096, 1024
T = B_DIM * S_DIM          # 8192 tokens
TC = T // N_CORES          # 1024 tokens per core
E, BK = 8, 64              # experts, bottleneck
EB = E * BK                # 512 concatenated expert axis
P = 128
NTT = TC // P              # token tiles per core
KC = D // P                # D chunks
BC = EB // P               # bottleneck chunks
LBLK = 512                 # token block for the blocked phases
NLB = TC // LBLK
TPB = LBLK // P            # token tiles per block
SCALE = 0.5
N_WARM = 11                # PE warm-up matmuls during initial DMA wait

F32 = mybir.dt.float32
BF16 = mybir.dt.bfloat16
AL = mybir.AluOpType
ACTF = mybir.ActivationFunctionType
AX = mybir.AxisListType

_BUILD_CACHE = {}


def _build(include_bd: bool, include_bu: bool, reps: int = 1):
    key = (include_bd, include_bu, reps)
    if key in _BUILD_CACHE:
        return _BUILD_CACHE[key]

    nc = bacc.Bacc("TRN2", target_bir_lowering=False, debug=False,
                   num_devices=N_CORES)
    # chunk-pair packed: [4, 128, 2048], [j,:,0:1024]=chunk 2j rows
    xh_d = nc.dram_tensor("xh", [KC // 2, P, 2 * TC], BF16,
                          kind="ExternalInput").ap()
    xl_d = nc.dram_tensor("xl", [KC // 2, P, 2 * TC], BF16,
                          kind="ExternalInput").ap()
    # Wd chunk-quads: [2, 128, 2048] where [q, :, i*512:] is D-chunk 4q+i
    wd_d = nc.dram_tensor("wd", [2, P, 4 * EB], BF16,
                          kind="ExternalInput").ap()
    # Wu chunk-pairs: [2, 128, 2048] where [q, :, i*1024:] is EB-chunk 2q+i
    wu_d = nc.dram_tensor("wu", [2, P, 2 * D], BF16,
                          kind="ExternalInput").ap()
    # [wgh | wgl] stacked on the stationary free dim: [P, KC, 16]
    wghl_d = nc.dram_tensor("wghl", [P, KC * 2 * E], BF16,
                            kind="ExternalInput").ap()
    id_d = nc.dram_tensor("ident", [P, P], F32, kind="ExternalInput").ap()
    idb_d = nc.dram_tensor("identb", [P, P], BF16, kind="ExternalInput").ap()
    eb_d = nc.dram_tensor("eblk", [E, EB], BF16, kind="ExternalInput").ap()
    if include_bd:
        bd_d = nc.dram_tensor("bd", [P, BC], F32, kind="ExternalInput").ap()
    if include_bu:
        bu_d = nc.dram_tensor("bu", [E, D], BF16, kind="ExternalInput").ap()
    out_d = nc.dram_tensor("out", [TC, D], BF16, kind="ExternalOutput").ap()

    with tile.TileContext(nc) as tc, ExitStack() as ctx:
        wpool = ctx.enter_context(tc.tile_pool(name="weights", bufs=1))
        hgpool = ctx.enter_context(tc.tile_pool(name="hg", bufs=6))
        gpool = ctx.enter_context(tc.tile_pool(name="gates", bufs=2))
        g2bpool = ctx.enter_context(tc.tile_pool(name="g2b", bufs=NTT))
        opool = ctx.enter_context(tc.tile_pool(name="osb", bufs=3))

        # PSUM: ht 5 banks + sml 1 (lt / gating / GB rotate) + ops 2 = 8
        ht_ps_pool = ctx.enter_context(
            tc.tile_pool(name="htps", bufs=5, space="PSUM"))
        s_ps_pool = ctx.enter_context(
            tc.tile_pool(name="sps", bufs=1, space="PSUM"))
        o_ps_pool = ctx.enter_context(
            tc.tile_pool(name="ops", bufs=2, space="PSUM"))

        # Everything streams on the sync HWDGE ring in chained FIFO
        # order (ordering-only deps so the scheduler can't shuffle it).
        # wghl first (gates the router chase), big 512KB inputs next,
        # the gating/expert constants just before wu (needed ~15us in).
        ident = wpool.tile([P, P], F32, tag="ident")
        identb = wpool.tile([P, P], BF16, tag="identb")
        eblk = wpool.tile([E, EB], BF16, tag="eblk")
        wghl_sb = wpool.tile([P, KC * 2 * E], BF16, tag="wghl")
        wdq = [wpool.tile([P, 4 * EB], BF16, tag=f"wd{q}", name=f"wd{q}")
               for q in range(2)]
        xh2 = [wpool.tile([P, 2 * TC], BF16, tag=f"xh{j}", name=f"xh{j}")
               for j in range(KC // 2)]
        xl2 = [wpool.tile([P, 2 * TC], BF16, tag=f"xl{j}", name=f"xl{j}")
               for j in range(KC // 2)]
        wuq = [wpool.tile([P, 2 * D], BF16, tag=f"wu{q}", name=f"wu{q}")
               for q in range(2)]
        prev_dma = [None]

        def big_dma(dst, src):
            h = nc.sync.dma_start(dst, src)
            if prev_dma[0] is not None:
                add_dep_helper(h.ins, prev_dma[0].ins, sync=False,
                               reason="sync ring FIFO order")
            prev_dma[0] = h

        big_dma(wghl_sb[:], wghl_d)
        big_dma(xh2[0][:], xh_d[0])
        big_dma(xh2[1][:], xh_d[1])
        big_dma(wdq[0][:], wd_d[0])
        big_dma(xh2[2][:], xh_d[2])
        big_dma(xh2[3][:], xh_d[3])
        big_dma(wdq[1][:], wd_d[1])
        for j in range(KC // 2):
            big_dma(xl2[j][:], xl_d[j])
        big_dma(ident[:], id_d)
        big_dma(identb[:], idb_d)
        big_dma(eblk[:], eb_d)
        if include_bd:
            bd_sb = wpool.tile([P, BC], F32, tag="bd")
            big_dma(bd_sb[:], bd_d)
        if include_bu:
            bu_sb = wpool.tile([E, D], BF16, tag="bu")
            big_dma(bu_sb[:], bu_d)
        for q in range(2):
            big_dma(wuq[q][:], wu_d[q])

        def xh_c(c):
            return xh2[c // 2][:, (c % 2) * TC:(c % 2 + 1) * TC]

        def xl_c(c):
            return xl2[c // 2][:, (c % 2) * TC:(c % 2 + 1) * TC]

        def wd_slice(c, k):
            base = (c % 4) * EB + k * P
            return wdq[c // 4][:, base:base + P]

        def wu_slice(k, h):
            base = (k % 2) * D + h * 512
            return wuq[k // 2][:, base:base + 512]

        # PE warm-up / HAM fillers
        warm_src = wpool.tile([P, EB], BF16, tag="warmsrc")
        nc.vector.memset(warm_src[:], 0.001)
        # preload the sigmoid ACT table while ACT is idle so the 1.28us
        # ACT_TABLE_LOAD never lands on the gating critical path
        sigwarm = wpool.tile([P, 1], F32, tag="sigwarm")
        nc.scalar.activation(sigwarm[:], warm_src[:, 0:1], ACTF.Sigmoid)
        warm_ps = o_ps_pool.tile([P, EB], F32, tag="ops")
        for i in range(N_WARM):
            nc.tensor.matmul(warm_ps[:], warm_src[:, 0:P], warm_src[:],
                             start=(i == 0), stop=(i == N_WARM - 1))

        def flr(n=1):
            for _ in range(n):
                nc.tensor.matmul(warm_ps[:], warm_src[:, 0:P], warm_src[:],
                                 start=True, stop=True)

        def mm(out, lhsT, rhs, start, stop):
            nc.tensor.matmul(out, lhsT, rhs, start=start, stop=stop,
                             skip_group_check=True)

        for rep in range(reps):
            # --- chase the xh stream: router terms 1+2 (one stacked
            # [128,16] stationary: rows 0:8 = xh@wgh, 8:16 = xh@wgl,
            # summed after the gating transpose), GEMM1 k=0 ---
            # lt lives in ONE psum bank: blk0 rows 0:16, blk1 rows 32:48
            lt_ps = s_ps_pool.tile([48, LBLK], F32, tag="sml", name="ltps")
            lt_v = [lt_ps[0:2 * E, :], lt_ps[32:32 + 2 * E, :]]
            lt_v8 = [lt_ps[0:E, :], lt_ps[32:32 + E, :]]
            ht_k0 = [ht_ps_pool.tile([P, LBLK], F32, tag="htps",
                                     name=f"htk0b{blk}") for blk in range(NLB)]
            for c in range(KC):
                for blk in range(NLB):
                    cols = bass.ts(blk, LBLK)
                    mm(lt_v[blk], wghl_sb[:, c * 16:c * 16 + 16],
                       xh_c(c)[:, cols], start=(c == 0), stop=False)
                    mm(ht_k0[blk][:], wd_slice(c, 0), xh_c(c)[:, cols],
                       start=(c == 0), stop=(c == KC - 1))

            # --- chase the xl stream: router term 3 (adds onto rows 0:8),
            # interleaved with HT passes so the PE head never blocks on
            # the xl pair cadence ---
            def t3_chunk(c):
                for blk in range(NLB):
                    mm(lt_v8[blk], wghl_sb[:, c * 16:c * 16 + E],
                       xl_c(c)[:, bass.ts(blk, LBLK)],
                       start=False, stop=(c == KC - 1))

            def ht_pass(blk, k, after=None):
                cols = bass.ts(blk, LBLK)
                ht_ps = ht_ps_pool.tile([P, LBLK], F32, tag="htps")
                for c in range(KC):
                    h = nc.tensor.matmul(ht_ps[:], wd_slice(c, k),
                                         xh_c(c)[:, cols], start=(c == 0),
                                         stop=(c == KC - 1),
                                         skip_group_check=True)
                    if c == 0 and after is not None:
                        add_dep_helper(h.ins, after.ins, sync=False,
                                       reason="fill gating-chain window")
                return ht_ps

            for c in range(KC):
                t3_chunk(c)
            lt_sb = []
            for blk in range(NLB):
                t = gpool.tile([2 * E, LBLK], F32, tag="ltsb",
                               name=f"ltsb{blk}")
                # parallel engines so both copies land together
                if blk == 0:
                    nc.scalar.copy(t[:], lt_v[blk])
                else:
                    nc.vector.tensor_copy(t[:], lt_v[blk])
                lt_sb.append(t)

            # --- gating, batched: all 8 token tiles in one [128, 64] chain.
            # g = 0.5 * sigmoid(2l - m1 - m2) * (l >= m2)  — exactly the
            # top-2 softmax (x0.5), two ACT/DVE ops per STEP not per tile.
            g2_all = gpool.tile([E, NTT, P], BF16, tag="g2t", name="g2all")
            g2ts = [g2_all[:, blk * TPB:(blk + 1) * TPB, :]
                    for blk in range(NLB)]
            l16_ps = s_ps_pool.tile([P, NTT, 2 * E], F32, tag="sml",
                                    name="l16ps")
            for blk in range(NLB):
                for bo in range(TPB):
                    nc.tensor.transpose(l16_ps[:, blk * TPB + bo, :],
                                        lt_sb[blk][:, bass.ts(bo, P)],
                                        ident[0:2 * E, 0:2 * E])
            l16 = gpool.tile([P, NTT, 2 * E], F32, tag="l16")
            nc.scalar.copy(l16[:], l16_ps[:])
            l_all = gpool.tile([P, NTT, E], F32, tag="lall")
            nc.vector.tensor_tensor(l_all[:], l16[:, :, 0:E],
                                    l16[:, :, E:2 * E], op=AL.add)
            m1 = gpool.tile([P, NTT], F32, tag="m1")
            nc.vector.tensor_reduce(m1[:], l_all[:], AX.X, AL.max)
            mask1 = gpool.tile([P, NTT, E], F32, tag="mask1")
            nc.vector.tensor_tensor(
                mask1[:], l_all[:],
                m1[:, :, None].broadcast_to([P, NTT, E]), op=AL.is_ge)
            lm = gpool.tile([P, NTT, E], F32, tag="lm")
            nc.vector.scalar_tensor_tensor(
                lm[:], mask1[:], -1e30, l_all[:], op0=AL.mult, op1=AL.add)
            m2 = gpool.tile([P, NTT], F32, tag="m2")
            nc.vector.tensor_reduce(m2[:], lm[:], AX.X, AL.max)
            negs = gpool.tile([P, NTT], F32, tag="negs")
            nc.vector.scalar_tensor_tensor(
                negs[:], m1[:], -1.0, m2[:], op0=AL.mult, op1=AL.subtract)
            mask2 = gpool.tile([P, NTT, E], F32, tag="mask2")
            nc.vector.tensor_tensor(
                mask2[:], l_all[:],
                m2[:, :, None].broadcast_to([P, NTT, E]), op=AL.is_ge)
            z = gpool.tile([P, NTT, E], F32, tag="z")
            nc.vector.scalar_tensor_tensor(
                z[:], l_all[:], 2.0,
                negs[:, :, None].broadcast_to([P, NTT, E]),
                op0=AL.mult, op1=AL.add)
            sig = gpool.tile([P, NTT, E], F32, tag="sig")
            nc.scalar.activation(sig[:], z[:], ACTF.Sigmoid)
            g2b_all = g2bpool.tile([P, NTT, E], BF16, tag="g2b")
            nc.vector.scalar_tensor_tensor(
                g2b_all[:], sig[:], SCALE, mask2[:], op0=AL.mult, op1=AL.mult)

            def g2_transposes():
                g2_ps = s_ps_pool.tile([E, NTT, P], BF16, tag="sml",
                                       name="g2ps")
                for t in range(NTT):
                    nc.tensor.transpose(g2_ps[:, t, :], g2b_all[:, t, :],
                                        identb[:])
                nc.vector.tensor_copy(g2_all[:], g2_ps[:])

            def gb_hg(blk, k, ht_ps, hgs):
                gb_ps = s_ps_pool.tile([P, LBLK], F32, tag="sml")
                mm(gb_ps[:], eblk[:, bass.ts(k, P)], g2ts[blk],
                   start=True, stop=True)
                gbb = hgpool.tile([P, LBLK], BF16, tag="gbb")
                nc.scalar.copy(gbb[:], gb_ps[:])
                hg_k = hgpool.tile([P, LBLK], BF16, tag="hg",
                                   name=f"hg{blk}_{k}")
                if include_bd:
                    r_k = hgpool.tile([P, LBLK], BF16, tag="relu")
                    nc.scalar.activation(r_k[:], ht_ps[:], ACTF.Relu,
                                         bias=bd_sb[:, k:k + 1])
                    nc.vector.tensor_tensor(hg_k[:], r_k[:], gbb[:],
                                            op=AL.mult)
                else:
                    # HG = max(HT, 0) * GB in one DVE op
                    nc.vector.scalar_tensor_tensor(
                        hg_k[:], ht_ps[:], 0.0, gbb[:],
                        op0=AL.max, op1=AL.mult)
                hgs.append(hg_k)

            def gemm2(blk, hgs):
                for bo in range(TPB):
                    tix = blk * TPB + bo
                    tok = bass.ts(bo, P)
                    o_sb = opool.tile([P, D], BF16, tag="osb")
                    for h in range(2):
                        o_ps = o_ps_pool.tile([P, 512], F32, tag="ops")
                        n_b = BC + (1 if include_bu else 0)
                        for k in range(BC):
                            mm(o_ps[:], hgs[k][:, tok], wu_slice(k, h),
                               start=(k == 0), stop=(k == n_b - 1))
                        if include_bu:
                            mm(o_ps[:], g2_all[:, blk * TPB + bo, :],
                               bu_sb[:, bass.ts(h, 512)],
                               start=False, stop=True)
                        if h == 0:
                            nc.vector.tensor_copy(o_sb[:, 0:512], o_ps[:])
                        else:
                            nc.scalar.copy(o_sb[:, 512:1024], o_ps[:])
                        if blk == NLB - 1 and bo == TPB - 1:
                            # tail tile: stream each half as soon as its
                            # copy lands, on the idle sync ring
                            nc.sync.dma_start(
                                out_d[bass.ts(tix, P), bass.ts(h, 512)],
                                o_sb[:, bass.ts(h, 512)])
                    if not (blk == NLB - 1 and bo == TPB - 1):
                        nc.scalar.dma_start(out_d[bass.ts(tix, P), :],
                                            o_sb[:])

            # PE order: remaining HT passes fill time while gating chains
            # run, then the g2 transposes land stall-free; ht psum bufs
            # recycle via gb_hg.
            hgs0, hgs1 = [], []
            ht1 = ht_pass(0, 1)
            ht2 = ht_pass(0, 2)
            ht3 = ht_pass(0, 3)
            g2_transposes()
            gb_hg(0, 0, ht_k0[0], hgs0)
            gb_hg(1, 0, ht_k0[1], hgs1)
            gb_hg(0, 1, ht1, hgs0)
            htb1 = ht_pass(1, 1)
            gb_hg(0, 2, ht2, hgs0)
            htb2 = ht_pass(1, 2)
            gb_hg(0, 3, ht3, hgs0)
            htb3 = ht_pass(1, 3)
            gemm2(0, hgs0)
            gb_hg(1, 1, htb1, hgs1)
            gb_hg(1, 2, htb2, hgs1)
            gb_hg(1, 3, htb3, hgs1)
            gemm2(1, hgs1)

    nc.compile()
    _BUILD_CACHE[key] = nc
    return nc


def _split_bf16(a):
    hi = a.astype(ml_dtypes.bfloat16)
    lo = (a - hi.astype(np.float32)).astype(ml_dtypes.bfloat16)
    return hi, lo


def _pair_pack(a, npair, rows, width):
    """[npair*2*rows, width] -> [npair, rows, 2*width] chunk-pair packing"""
    return np.ascontiguousarray(
        a.reshape(npair, 2, rows, width).transpose(0, 2, 1, 3)
        .reshape(npair, rows, 2 * width))


def kernel(x, w_gate, w_noise, Wd, bd, Wu, bu, reps: int = 1):
    x = np.ascontiguousarray(np.asarray(x, dtype=np.float32))
    assert x.shape == (B_DIM, S_DIM, D), x.shape
    wg = np.ascontiguousarray(np.asarray(w_gate, dtype=np.float32))
    Wd = np.asarray(Wd, dtype=np.float32)
    Wu = np.asarray(Wu, dtype=np.float32)
    bd = np.asarray(bd, dtype=np.float32)
    bu = np.asarray(bu, dtype=np.float32)

    include_bd = bool(np.any(bd))
    include_bu = bool(np.any(bu))
    nc = _build(include_bd, include_bu, reps)

    xf = x.reshape(T, D)
    xh, xl = _split_bf16(xf)
    xht_full = np.ascontiguousarray(xh.T)   # [D, T]
    xlt_full = np.ascontiguousarray(xl.T)
    wgh, wgl = _split_bf16(wg)
    # [P, KC*16] packed: wghl_p[p, c*16+e] = wgh[c*128+p, e], +8 -> wgl
    wghl_p = np.ascontiguousarray(
        np.concatenate([wgh.reshape(KC, P, E), wgl.reshape(KC, P, E)],
                       axis=2).transpose(1, 0, 2).reshape(P, KC * 2 * E))
    wd_all = Wd.transpose(1, 0, 2).reshape(D, EB).astype(ml_dtypes.bfloat16)
    # chunk quads [2, 128, 2048]: [q, :, i*512:(i+1)*512] = chunk 4q+i
    wd4 = np.ascontiguousarray(
        wd_all.reshape(2, 4, P, EB).transpose(0, 2, 1, 3)
        .reshape(2, P, 4 * EB))
    wu_flat = Wu.reshape(EB, D).astype(ml_dtypes.bfloat16)
    wu2 = _pair_pack(wu_flat, 2, P, D)       # [2, 128, 2048]
    ident = np.eye(P, dtype=np.float32)
    identb = np.eye(P, dtype=ml_dtypes.bfloat16)
    eblk = np.kron(np.eye(E, dtype=np.float32),
                   np.ones((1, BK), dtype=np.float32)
                   ).astype(ml_dtypes.bfloat16)  # [E, EB]

    shared = dict(wd=wd4, wu=wu2, wghl=wghl_p, ident=ident,
                  identb=identb, eblk=eblk)
    if include_bd:
        # [P, BC] partition-major per chunk: bd_sb[p, k] = bd_flat[128k+p]
        shared["bd"] = np.ascontiguousarray(
            bd.reshape(EB)[np.arange(P)[:, None] + P * np.arange(BC)[None]])
    if include_bu:
        shared["bu"] = np.ascontiguousarray(bu.astype(ml_dtypes.bfloat16))

    in_maps = []
    for c in range(N_CORES):
        sl = slice(c * TC, (c + 1) * TC)
        in_maps.append(dict(xh=_pair_pack(xht_full[:, sl], KC // 2, P, TC),
                            xl=_pair_pack(xlt_full[:, sl], KC // 2, P, TC),
                            **shared))
    kernel.last_in_maps = in_maps
    res = run_bass_kernel_spmd(nc, in_maps, core_ids=list(range(N_CORES)))
    out = np.concatenate([np.asarray(res.results[c]["out"])
                          for c in range(N_CORES)], axis=0)
    return out.astype(np.float32).reshape(B_DIM, S_DIM, D)
